# revision 54
# baseline (speedup 1.0000x reference)
"""Trainium2 Bass kernel for nn_Diffusion_ViTCA_NCA (ViT-Cellular-Automata, 6 NCA steps).

Data-parallel over batch B=8 across 8 NeuronCores (1 sample/core), fully
SBUF-resident state, feature-major half-stacked layout
[128 partitions = (2 row-halves x 64 channels), 2048 tokens-per-half].

v2 restructure vs baseline:
- conv3x3 pair + fc0 folded into one effective 3x3 conv (64 -> 128 hidden).
- block-diagonal [128]-contraction matmuls process both row-halves at once
  for qkv / out-proj / ff1 / ff2 / LN stats.
- norm0 global stats via activation accum_out (relu eviction) + DVE
  tensor_tensor_reduce (sum of squares); no separate sum matmuls.
- 1/std via ACT Ln+Exp (keeps one activation table set; no DVE reciprocal
  for LN, no Sqrt table reloads).
- attention q*k products as all-bf16 tensor_tensor (2x DVE mode); E*v
  products split DVE (half) / GPSIMD-Pool (half).
- PSUM evictions of q/k/v tiles on Pool; ch0 invariance via zeroed output
  rows in fc1/out/ff2 weights instead of restore copies.
"""

import os
import numpy as np
import ml_dtypes

# CoreSim's executor lacks Gelu; sim harnesses set this to validate the rest
# of the graph with Tanh substituted (never set in production use).
_GELU_SUBST_TANH = bool(os.environ.get('BASS_GELU_SUBST_TANH'))
# debug bisection toggles: comma list in BASS_SAFE, e.g. "pool,acc,poly,swap"
_SAFE = set(filter(None, os.environ.get('BASS_SAFE', '').split(',')))

C = 64; HID = 128; HEADS = 4; HD = 16; H = 64; W = 64; B = 8; STEPS = 6
MLP = 64; SCALE = HD ** -0.5
RH = 32            # image rows per half
PR = 34            # padded rows per half buffer
PITCH = 68         # padded col pitch
NH = RH * W        # 2048 tokens per half
NPAD = PR * PITCH  # 2312
EPS = 1e-5
BF16 = ml_dtypes.bfloat16

_BUILD_CACHE = {}


def _pad_state(xb):
    """xb [H,W,C] -> padded half-stacked state [128, PR*PITCH] fp32."""
    cf = np.transpose(np.asarray(xb, np.float32), (2, 0, 1))   # [C,H,W]
    buf = np.zeros((128, PR, PITCH), np.float32)
    for s in range(2):
        buf[64 * s:64 * s + 64, 1:33, 2:66] = cf[:, RH * s:RH * s + RH, :]
    buf[:, 1:33, 1] = buf[:, 1:33, 3]
    buf[:, 1:33, 66] = buf[:, 1:33, 64]
    buf[0:64, 0, :] = buf[0:64, 2, :]
    buf[64:128, 33, :] = buf[64:128, 31, :]
    buf[0:64, 33, :] = buf[64:128, 1, :]
    buf[64:128, 0, :] = buf[0:64, 32, :]
    return buf.reshape(128, NPAD)


def _bd(w):
    """[64out,64in] -> block-diag lhsT [128, 128] (both halves)."""
    z = np.zeros((128, 128), np.float32)
    z[0:64, 0:64] = w.T
    z[64:128, 64:128] = w.T
    return z


def _pack_weights(ip):
    f32 = np.float32
    w = {}
    assert np.allclose(np.asarray(ip['norm0_w'], f32), 1.0), "norm0_w!=1 unsupported"
    assert np.allclose(np.asarray(ip['norm0_b'], f32), 0.0), "norm0_b!=0 unsupported"
    for _bn in ('p0_b', 'p1_b', 'fc0_b', 'ff1_b', 'ln1_b', 'ln2_b'):
        assert np.allclose(np.asarray(ip[_bn], f32), 0.0), _bn + "!=0 unsupported"

    # ---- folded conv+fc0: Weff[j, i, di, dj] ----
    p0, p1 = np.asarray(ip['p0_w'], f32), np.asarray(ip['p1_w'], f32)
    fc0 = np.asarray(ip['fc0_w'], f32)      # [HID, 3C] = [x | y1 | y2]
    fc0x, fc0y1, fc0y2 = fc0[:, 0:64], fc0[:, 64:128], fc0[:, 128:192]
    cw = np.zeros((64, 9, 128), f32)        # [in-ch, offset, hid-out]
    for i in range(3):
        for j in range(3):
            o = 3 * i + j
            weff = fc0y1 @ p0[:, :, i, j] + fc0y2 @ p1[:, :, i, j]  # [HID, in]
            if i == 1 and j == 1:
                weff = weff + fc0x
            cw[:, o, :] = weff.T
    w['cweff'] = np.concatenate([cw, cw], 0).reshape(128, 9 * 128).astype(BF16)

    # ---- fc1 (ch0 of each half zeroed to keep input channel invariant) ----
    fc1 = np.asarray(ip['fc1_w'], f32).copy()
    fc1[0, :] = 0.0
    w['fc1t'] = fc1.T.copy().astype(BF16)                  # [HID, 64]
    rs = fc1.sum(1)
    w['fc1rs_row'] = np.concatenate([rs, rs]).reshape(1, 128).astype(f32)

    # ---- qkv block-diag (ln1 weight + q-scale folded) ----
    ln1w = np.asarray(ip['ln1_w'], f32)
    qkv = np.asarray(ip['qkv_w'], f32)
    qkv_eff = qkv * ln1w[None, :]
    qkv_eff[0:64] *= SCALE
    qbd = np.zeros((128, 384), f32)
    for c_ in range(3):
        qbd[:, 128 * c_:128 * c_ + 128] = _bd(qkv_eff[64 * c_:64 * c_ + 64])
    w['qkvbd'] = qbd.astype(BF16)

    # ---- out-proj block-diag, ch0 rows zeroed ----
    ow = np.asarray(ip['out_w'], f32).copy()
    ow[0, :] = 0.0
    w['outwbd'] = _bd(ow).astype(BF16)
    outb = np.asarray(ip['out_b'], f32).copy()
    outb[0] = 0.0
    w['outb_row'] = np.concatenate([outb, outb]).reshape(1, 128).astype(BF16)

    # ---- ff block-diag ----
    ln2w = np.asarray(ip['ln2_w'], f32)
    ff1 = np.asarray(ip['ff1_w'], f32) * ln2w[None, :]
    w['ff1bd'] = _bd(ff1).astype(BF16)
    ff2 = np.asarray(ip['ff2_w'], f32).copy()
    ff2[0, :] = 0.0
    w['ff2bd'] = _bd(ff2).astype(BF16)
    ff2b = np.asarray(ip['ff2_b'], f32).copy()
    ff2b[0] = 0.0
    w['ff2b_row'] = np.concatenate([ff2b, ff2b]).reshape(1, 128).astype(BF16)

    # ---- attention patterns ----
    e_of_p = (np.arange(128) // 64) * 4 + ((np.arange(128) % 64) // 16)
    dots = np.zeros((128, 9, 72), f32)
    for o in range(9):
        dots[np.arange(128), o, o * 8 + e_of_p] = 1.0
    w['dotslhs'] = dots.reshape(128, 9 * 72).astype(BF16)
    z = np.zeros((72, 8), f32)
    z[np.arange(72), np.arange(72) % 8] = 1.0
    w['zlhs'] = z.astype(BF16)
    rb = np.zeros((8, 128), f32)
    rb[e_of_p, np.arange(128)] = 1.0
    w['rcpbl'] = rb.astype(BF16)
    eb = np.zeros((72, 9, 128), f32)
    for o in range(9):
        eb[o * 8 + e_of_p, o, np.arange(128)] = 1.0
    w['eblhs'] = eb.reshape(72, 9 * 128).astype(BF16)
    w['id128'] = np.eye(128, dtype=f32).astype(BF16)
    w['onesbd'] = _bd(np.ones((64, 64), f32)).astype(BF16)
    w['ones128f'] = np.ones((128, 1), f32)
    w['ones128'] = np.ones((128, 1), f32).astype(BF16)
    w['onesrow'] = np.ones((1, 512), f32).astype(BF16)
    sw128 = np.zeros((128, 128), f32)
    sw128[np.arange(128), (np.arange(128) + 64) % 128] = 1.0
    w['swap128'] = sw128.astype(BF16)
    return w


def _wspecs(dt):
    return {
        'cweff': ([128, 9 * 128], dt.bfloat16),
        'fc1t': ([128, 64], dt.bfloat16), 'fc1rs_row': ([1, 128], dt.float32),
        'qkvbd': ([128, 384], dt.bfloat16),
        'outwbd': ([128, 128], dt.bfloat16), 'outb_row': ([1, 128], dt.bfloat16),
        'ff1bd': ([128, 128], dt.bfloat16), 'ff2bd': ([128, 128], dt.bfloat16),
        'ff2b_row': ([1, 128], dt.bfloat16),
        'dotslhs': ([128, 9 * 72], dt.bfloat16), 'zlhs': ([72, 8], dt.bfloat16),
        'rcpbl': ([8, 128], dt.bfloat16), 'eblhs': ([72, 9 * 128], dt.bfloat16),
        'id128': ([128, 128], dt.bfloat16), 'onesbd': ([128, 128], dt.bfloat16),
        'ones128f': ([128, 1], dt.float32), 'onesrow': ([1, 512], dt.bfloat16),
        'ones128': ([128, 1], dt.bfloat16), 'swap128': ([128, 128], dt.bfloat16),
    }


def _build():
    if 'nc' in _BUILD_CACHE:
        return _BUILD_CACHE['nc']
    import concourse.bass as bass
    import concourse.bacc as bacc
    import concourse.tile as tile
    from concourse import mybir
    dt = mybir.dt
    ALU = mybir.AluOpType
    AF = mybir.ActivationFunctionType
    AX = mybir.AxisListType

    nc = bacc.Bacc("TRN2", target_bir_lowering=False)
    wspecs = _wspecs(dt)

    d_x0 = nc.dram_tensor('x0', [128, NPAD], dt.float32, kind='ExternalInput')
    d_mask = nc.dram_tensor('maskrep', [STEPS, 128, NH], dt.bfloat16, kind='ExternalInput')
    d_w = {k: nc.dram_tensor(k, list(s), d, kind='ExternalInput') for k, (s, d) in wspecs.items()}
    d_out = nc.dram_tensor('xout', [128, NH], dt.float32, kind='ExternalOutput')

    A = lambda h: h.ap()

    with tile.TileContext(nc) as tc:
        xpad = nc.alloc_sbuf_tensor('xpad', [128, NPAD], dt.float32)
        xpadB = nc.alloc_sbuf_tensor('xpadB', [128, NPAD], dt.bfloat16)
        maskS = nc.alloc_sbuf_tensor('maskS', [128, STEPS * NH], dt.bfloat16)
        sw = {k: nc.alloc_sbuf_tensor('w_' + k, list(s), d) for k, (s, d) in wspecs.items()}
        hd = nc.alloc_sbuf_tensor('hd', [128, 2 * NH], dt.bfloat16)
        hdsq = nc.alloc_sbuf_tensor('hdsq', [128, 1024], dt.bfloat16)
        hd2f = (nc.alloc_sbuf_tensor('hd2f', [128, 2 * NH], dt.bfloat16)
                if 'acc' in _SAFE else None)
        tB = nc.alloc_sbuf_tensor('tB', [128, NH], dt.bfloat16)
        t2B = nc.alloc_sbuf_tensor('t2B', [128, NH], dt.bfloat16)
        uB = nc.alloc_sbuf_tensor('uB', [128, NH], dt.bfloat16)
        varB = nc.alloc_sbuf_tensor('varB', [128, NH], dt.float32)
        lnB = nc.alloc_sbuf_tensor('lnB', [128, NH], dt.float32)
        invB = nc.alloc_sbuf_tensor('invB', [128, NH], dt.float32)
        dnegB = nc.alloc_sbuf_tensor('dnegB', [128, NH], dt.float32)
        ynegB = nc.alloc_sbuf_tensor('ynegB', [128, NH], dt.bfloat16)
        qS = nc.alloc_sbuf_tensor('qS', [128, NH], dt.bfloat16)
        kpad = nc.alloc_sbuf_tensor('kpad', [128, NPAD], dt.bfloat16)
        vpadA = nc.alloc_sbuf_tensor('vpadA', [128, NPAD], dt.bfloat16)
        Ebuf = nc.alloc_sbuf_tensor('Ebuf', [72, NH], dt.bfloat16)
        e1B = nc.alloc_sbuf_tensor('e1B', [72, NH], dt.bfloat16)
        Zb = nc.alloc_sbuf_tensor('Zb', [8, NH], dt.bfloat16)
        rcpb = nc.alloc_sbuf_tensor('rcpb', [128, NH], dt.float32)
        pB = nc.alloc_sbuf_tensor('pB', [128, 4 * NH], dt.bfloat16)
        ebS = nc.alloc_sbuf_tensor('ebS', [128, 2 * NH], dt.bfloat16)
        oS = nc.alloc_sbuf_tensor('oS', [128, NH], dt.bfloat16)
        gB = nc.alloc_sbuf_tensor('gB', [128, NH], dt.bfloat16)
        tmpd = nc.alloc_sbuf_tensor('tmpd', [128, 2 * NH], dt.bfloat16)
        sc = nc.alloc_sbuf_tensor('scal', [1, 16], dt.float32)
        sc2 = nc.alloc_sbuf_tensor('scal2', [128, 8], dt.float32)
        row2 = nc.alloc_sbuf_tensor('row2', [1, 2], dt.bfloat16)
        bc3 = nc.alloc_sbuf_tensor('bc3', [128, 2], dt.float32)
        mbrow = nc.alloc_sbuf_tensor('mbrow', [1, 128], dt.bfloat16)
        epsb = nc.alloc_sbuf_tensor('epsb', [128, 1], dt.float32)
        eps2b = nc.alloc_sbuf_tensor('eps2b', [128, 1], dt.float32)

        v3 = lambda h: A(h).rearrange('p (r c) -> p r c', r=PR)
        cv3 = lambda h: v3(h)[:, 1:33, 2:66]
        r3 = lambda ap, cols: ap[:, cols].rearrange('p (r c) -> p r c', c=64)

        nc.sync.dma_start(out=A(xpad), in_=A(d_x0))
        nc.sync.dma_start(out=A(maskS).rearrange('p (s n) -> p s n', s=STEPS),
                          in_=A(d_mask).rearrange('s p n -> p s n'))
        for k in wspecs:
            nc.sync.dma_start(out=A(sw[k]), in_=A(d_w[k]))
        nc.vector.memset(A(epsb), EPS)
        nc.vector.memset(A(eps2b), 4096.0 * EPS)
        nc.vector.memset(A(kpad), 0.0)
        nc.vector.memset(A(vpadA), 0.0)

        import contextlib
        stack = contextlib.ExitStack()
        p512 = stack.enter_context(tc.tile_pool(name='p512', bufs=4, space='PSUM'))
        p1k = stack.enter_context(tc.tile_pool(name='p1k', bufs=2, space='PSUM'))

        NT = NH // 512

        def stt(eng, out, in0, op0, scalar, op1, in1):
            eng.scalar_tensor_tensor(out=out, in0=in0, scalar=scalar, in1=in1,
                                     op0=op0, op1=op1)

        def swap_halo(buf, psrc):
            """Cross-half halo rows of a padded bf16 buffer via PE half-swap
            matmul (psrc: callable row -> rhs view). Writes buf rows 0 / 33."""
            if 'swap' in _SAFE:
                nc.sync.dma_start(out=v3(buf)[0:64, 33:34, :],
                                  in_=v3(buf)[64:128, 1:2, :])
                nc.sync.dma_start(out=v3(buf)[64:128, 0:1, :],
                                  in_=v3(buf)[0:64, 32:33, :])
                return
            hp1 = p512.tile([128, 136], dt.float32, tag='t512')
            nc.tensor.matmul(hp1[:, 0:68], A(sw['swap128']), psrc(1),
                             start=True, stop=True)
            nc.tensor.matmul(hp1[:, 68:136], A(sw['swap128']), psrc(32),
                             start=True, stop=True)
            nc.scalar.activation(v3(buf)[0:64, 33:34, :], hp1[0:64, 0:68], AF.Copy)
            nc.scalar.activation(v3(buf)[64:128, 0:1, :], hp1[64:128, 68:136], AF.Copy)

        for step in range(STEPS):
            mstep = A(maskS)[:, step * NH:(step + 1) * NH]

            # bf16 shadow: interior copy, then reflect borders + cross-half
            # halos rebuilt in bf16 (fp32 xpad borders are never maintained)
            nc.scalar.activation(A(xpadB), A(xpad), AF.Copy)
            # warm the sqrt activation table off the critical path; reading
            # xpadB anchors it inside this step (scale=0 keeps sqrt in-domain)
            nc.scalar.activation(A(sc)[0:1, 15:16], A(xpadB)[0:1, 0:1], AF.Sqrt,
                                 scale=0.0, bias=A(epsb)[0:1, :])
            nc.scalar.activation(v3(xpadB)[:, 1:33, 1:2], v3(xpad)[:, 1:33, 3:4], AF.Copy)
            nc.scalar.activation(v3(xpadB)[:, 1:33, 66:67], v3(xpad)[:, 1:33, 64:65], AF.Copy)
            nc.scalar.activation(v3(xpadB)[0:64, 0:1, :], v3(xpad)[0:64, 2:3, :], AF.Copy)
            nc.scalar.activation(v3(xpadB)[64:128, 33:34, :], v3(xpad)[64:128, 31:32, :], AF.Copy)
            swap_halo(xpadB, lambda r: v3(xpadB)[:, r:r + 1, :])

            # folded conv+fc0 (9 offsets, 64ch -> 128 hid) + relu -> hd;
            # relu eviction accumulates per-partition sums for norm0.
            for k in range(2):
                for s in range(2):
                    w_idx = 2 * k + s
                    hp = p1k.tile([128, 1024], dt.float32, tag='t1k')
                    for o in range(9):
                        di, dj = o // 3 - 1, o % 3 - 1
                        for t in range(2):
                            rs = 16 * k + 8 * t
                            rhs = v3(xpadB)[64 * s:64 * s + 64,
                                            1 + di + rs:1 + di + rs + 8, 2 + dj:66 + dj]
                            nc.tensor.matmul(hp[:, 512 * t:512 * t + 512],
                                             A(sw['cweff'])[64 * s:64 * s + 64,
                                                            o * 128:(o + 1) * 128],
                                             rhs, start=(o == 0), stop=(o == 8))
                    col = s * NH + 1024 * k
                    if 'acc' in _SAFE:
                        nc.scalar.activation(A(hd)[:, col:col + 1024], hp[:], AF.Relu)
                        nc.scalar.activation(A(hd2f)[:, col:col + 1024],
                                             A(hd)[:, col:col + 1024], AF.Square)
                    else:
                        # NOTE: vector.tensor_tensor_reduce wedges the device
                        # (NRT exec-unit unrecoverable) — ACT Square+accum_out
                        # computes the sum-of-squares instead.
                        nc.scalar.activation(A(hd)[:, col:col + 1024], hp[:], AF.Relu,
                                             accum_out=A(sc2)[:, w_idx:w_idx + 1])
                        nc.scalar.activation(A(hdsq), A(hd)[:, col:col + 1024],
                                             AF.Square,
                                             accum_out=A(sc2)[:, 4 + w_idx:5 + w_idx])

            # norm0 global stats
            s1, s2 = A(sc)[:, 0:1], A(sc)[:, 1:2]
            mean, e2 = A(sc)[:, 2:3], A(sc)[:, 3:4]
            m2, var = A(sc)[:, 4:5], A(sc)[:, 5:6]
            stdv, istd, nm = A(sc)[:, 6:7], A(sc)[:, 7:8], A(sc)[:, 8:9]
            if 'acc' in _SAFE:
                sAa = p512.tile([1, 512], dt.float32, tag='t512')
                sBb = p512.tile([1, 512], dt.float32, tag='t512')
                for t in range(8):
                    nc.tensor.matmul(sAa[:], A(sw['ones128']),
                                     A(hd)[:, 512 * t:512 * t + 512],
                                     start=(t == 0), stop=(t == 7))
                for t in range(8):
                    nc.tensor.matmul(sBb[:], A(sw['ones128']),
                                     A(hd2f)[:, 512 * t:512 * t + 512],
                                     start=(t == 0), stop=(t == 7))
                nc.vector.tensor_reduce(s1, sAa[:], AX.X, ALU.add)
                nc.vector.tensor_reduce(s2, sBb[:], AX.X, ALU.add)
            else:
                sA = p512.tile([1, 8], dt.float32, tag='t512')
                nc.tensor.matmul(sA[:], A(sw['ones128f']), A(sc2)[:], start=True, stop=True)
                nc.vector.tensor_reduce(s1, sA[:, 0:4], AX.X, ALU.add)
                nc.vector.tensor_reduce(s2, sA[:, 4:8], AX.X, ALU.add)
            NTOT = 1.0 / (2 * NH * 128)
            nc.vector.tensor_scalar_mul(mean, s1, NTOT)
            nc.vector.tensor_scalar_mul(e2, s2, NTOT)
            stt(nc.vector, m2, mean, ALU.bypass, 0.0, ALU.mult, mean)
            stt(nc.vector, var, e2, ALU.bypass, 0.0, ALU.subtract, m2)
            nc.scalar.activation(stdv, var, AF.Sqrt, bias=A(epsb)[0:1, :])
            nc.vector.reciprocal_approx_fast(istd, stdv)
            nc.vector.tensor_scalar_mul(nm, mean, -1.0)
            nc.scalar.copy(A(row2)[:, 0:1], istd)
            nc.vector.tensor_scalar_mul(A(mbrow), A(sw['fc1rs_row']), nm)
            bcp = p512.tile([128, 2], dt.float32, tag='t512')
            nc.tensor.matmul(bcp[:, 0:1], A(sw['onesrow'])[:, 0:128], A(row2)[:, 0:1],
                             start=True, stop=True)
            nc.scalar.copy(A(bc3)[:, 0:1], bcp[:, 0:1])

            # fc1 (+ global-mean correction row) -> dx ; x += (dx*istd)*mask
            for k in range(2):
                dp = p1k.tile([128, 1024], dt.float32, tag='t1k')
                for t in range(2):
                    colh = 1024 * k + 512 * t
                    for half in range(2):
                        osl = (slice(64 * half, 64 * half + 64),
                               slice(512 * t, 512 * t + 512))
                        nc.tensor.matmul(dp[osl[0], osl[1]], A(sw['fc1t']),
                                         A(hd)[:, half * NH + colh:half * NH + colh + 512],
                                         start=True, stop=False)
                        nc.tensor.matmul(dp[osl[0], osl[1]],
                                         A(mbrow)[:, 64 * half:64 * half + 64],
                                         A(sw['onesrow']), start=False, stop=True)
                cs = slice(1024 * k, 1024 * k + 1024)
                tmp = A(dnegB)[:, cs]
                stt(nc.vector, tmp, dp[:], ALU.mult, A(bc3)[:, 0:1], ALU.mult, mstep[:, cs])
                rows = slice(1 + 16 * k, 1 + 16 * k + 16)
                stt(nc.vector, v3(xpad)[:, rows, 2:66], r3(A(dnegB), cs),
                    ALU.bypass, 0.0, ALU.add, v3(xpad)[:, rows, 2:66])

            def layernorm_to(dst):
                nc.scalar.activation(A(tB), cv3(xpad), AF.Copy)
                if 'pool' in _SAFE:
                    nc.scalar.activation(A(t2B), A(tB), AF.Square)
                else:
                    # x^2 on Pool straight from fp32 state (parallel with tB)
                    for kk2 in range(2):
                        c2 = slice(1024 * kk2, 1024 * kk2 + 1024)
                        rw = slice(1 + 16 * kk2, 1 + 16 * kk2 + 16)
                        nc.gpsimd.tensor_tensor(out=r3(A(t2B), c2),
                                                in0=v3(xpad)[:, rw, 2:66],
                                                in1=v3(xpad)[:, rw, 2:66], op=ALU.mult)
                for kk in range(2):
                    cs2 = slice(1024 * kk, 1024 * kk + 1024)
                    mu = p1k.tile([128, 1024], dt.float32, tag='t1k')
                    sq = p1k.tile([128, 1024], dt.float32, tag='t1k')
                    for tt in range(2):
                        nsl = slice(1024 * kk + 512 * tt, 1024 * kk + 512 * tt + 512)
                        osl = slice(512 * tt, 512 * tt + 512)
                        nc.tensor.matmul(mu[:, osl], A(sw['onesbd']),
                                         A(tB)[:, nsl], start=True, stop=True)
                        nc.tensor.matmul(sq[:, osl], A(sw['onesbd']),
                                         A(t2B)[:, nsl], start=True, stop=True)
                    nc.scalar.activation(A(uB)[:, cs2], mu[:], AF.Square)
                    stt(nc.vector, A(varB)[:, cs2], sq[:], ALU.mult, 64.0,
                        ALU.subtract, A(uB)[:, cs2])
                    nc.scalar.activation(A(lnB)[:, cs2], A(varB)[:, cs2], AF.Sqrt,
                                         bias=A(eps2b))
                    nc.vector.reciprocal_approx_fast(A(invB)[:, cs2], A(lnB)[:, cs2])
                    rows2 = slice(1 + 16 * kk, 1 + 16 * kk + 16)
                    stt(nc.vector, r3(A(dnegB), cs2), v3(xpad)[:, rows2, 2:66],
                        ALU.mult, 64.0, ALU.subtract, mu[:].rearrange('p (r c) -> p r c', c=64))
                    if 'pool' in _SAFE:
                        stt(nc.vector, A(dst)[:, cs2], A(dnegB)[:, cs2],
                            ALU.bypass, 0.0, ALU.mult, A(invB)[:, cs2])
                    else:
                        nc.gpsimd.tensor_tensor(out=A(dst)[:, cs2],
                                                in0=A(dnegB)[:, cs2],
                                                in1=A(invB)[:, cs2], op=ALU.mult)

            # LN1 + qkv (block-diagonal, both halves per matmul)
            layernorm_to(ynegB)
            for k in range(NT):
                nsl = slice(512 * k, 512 * k + 512)
                qp = p512.tile([128, 512], dt.float32, tag='t512')
                kp = p512.tile([128, 512], dt.float32, tag='t512')
                vp = p512.tile([128, 512], dt.float32, tag='t512')
                nc.tensor.matmul(qp[:], A(sw['qkvbd'])[:, 0:128],
                                 A(ynegB)[:, nsl], start=True, stop=True)
                nc.tensor.matmul(kp[:], A(sw['qkvbd'])[:, 128:256],
                                 A(ynegB)[:, nsl], start=True, stop=True)
                nc.tensor.matmul(vp[:], A(sw['qkvbd'])[:, 256:384],
                                 A(ynegB)[:, nsl], start=True, stop=True)
                rr = slice(1 + 8 * k, 1 + 8 * k + 8)
                pr3 = lambda ps: ps[:].rearrange('p (r c) -> p r c', c=64)
                nc.scalar.activation(A(qS)[:, nsl], qp[:], AF.Copy)
                nc.scalar.activation(v3(kpad)[:, rr, 2:66], pr3(kp), AF.Copy)
                nc.scalar.activation(v3(vpadA)[:, rr, 2:66], pr3(vp), AF.Copy)
            # cross-half halos for k/v via PE half-swap (no DMA latency)
            swap_halo(kpad, lambda r: v3(kpad)[:, r:r + 1, :])
            swap_halo(vpadA, lambda r: v3(vpadA)[:, r:r + 1, :])

            # dots -> pd psum; exp via 2nd-order Taylor (|logit| < 0.16):
            # e = 1 + z + z^2/2 computed as (z * (0.5 z + 1)) + 1
            for k in range(2):
                cs = slice(1024 * k, 1024 * k + 1024)
                pd = p1k.tile([72, 1024], dt.float32, tag='t1k')
                for o in range(9):
                    di, dj = o // 3 - 1, o % 3 - 1
                    rows = slice(1 + di + 16 * k, 1 + di + 16 * k + 16)
                    po = (o % 2) * 1024
                    pcs = slice(po, po + 1024)
                    nc.vector.tensor_tensor(out=r3(A(tmpd), pcs), in0=r3(A(qS), cs),
                                            in1=v3(kpad)[:, rows, 2 + dj:66 + dj],
                                            op=ALU.mult)
                    for t in range(2):
                        fs = slice(512 * t, 512 * t + 512)
                        nc.tensor.matmul(pd[:, fs], A(sw['dotslhs'])[:, o * 72:(o + 1) * 72],
                                         A(tmpd)[:, po + 512 * t:po + 512 * t + 512],
                                         start=(o == 0), stop=(o == 8))
                if 'poly' in _SAFE:
                    nc.scalar.activation(A(Ebuf)[:, cs], pd[:], AF.Exp)
                else:
                    nc.scalar.activation(A(e1B)[:, cs], pd[:], AF.Copy,
                                         scale=0.5, bias=1.0)
                    nc.vector.tensor_tensor(out=A(Ebuf)[:, cs], in0=pd[:],
                                            in1=A(e1B)[:, cs], op=ALU.mult)
                    nc.vector.tensor_scalar(out=A(Ebuf)[:, cs], in0=A(Ebuf)[:, cs],
                                            scalar1=1.0, scalar2=None, op0=ALU.add)
                # Z and its reciprocal broadcast (needed only at the oS stage)
                zp = p1k.tile([8, 1024], dt.float32, tag='t1k')
                for t in range(2):
                    fs = slice(512 * t, 512 * t + 512)
                    nc.tensor.matmul(zp[:, fs], A(sw['zlhs']),
                                     A(Ebuf)[:, 1024 * k + 512 * t:1024 * k + 512 * t + 512],
                                     start=True, stop=True)
                nc.scalar.activation(A(Zb)[:, cs], zp[:], AF.Copy)
                zbc = p1k.tile([128, 1024], dt.float32, tag='t1k')
                for t in range(2):
                    fs = slice(512 * t, 512 * t + 512)
                    nc.tensor.matmul(zbc[:, fs], A(sw['rcpbl']),
                                     A(Zb)[:, 1024 * k + 512 * t:1024 * k + 512 * t + 512],
                                     start=True, stop=True)
                nc.vector.reciprocal_approx_fast(A(rcpb)[:, cs], zbc[:])

            # o = (sum_o Ebcast_o * v_shift_o) * rcp — k-halves interleaved:
            # E*v products stream on DVE (k=0) and Pool (k=1) in parallel
            op_ps0 = p1k.tile([128, 1024], dt.float32, tag='t1k')
            op_ps1 = p1k.tile([128, 1024], dt.float32, tag='t1k')
            op_ps = [op_ps0, op_ps1]
            pend = {}   # (k,t) -> pB offset of the previous product (PE pipelining)
            for o in range(9):
                di, dj = o // 3 - 1, o % 3 - 1
                for k in range(2):
                    for t in range(2):
                        ebp = p512.tile([128, 512], dt.float32, tag='t512')
                        nc.tensor.matmul(ebp[:], A(sw['eblhs'])[:, o * 128:(o + 1) * 128],
                                         A(Ebuf)[:, 1024 * k + 512 * t:1024 * k + 512 * t + 512],
                                         start=True, stop=True)
                        po = (2 * k + t) * 2048 + (o % 4) * 512
                        rows = slice(1 + di + 16 * k + 8 * t, 1 + di + 16 * k + 8 * t + 8)
                        vview = v3(vpadA)[:, rows, 2 + dj:66 + dj]
                        if k == 0 or 'pool' in _SAFE:
                            # DVE multiplies straight from PSUM
                            stt(nc.vector, r3(A(pB), slice(po, po + 512)),
                                ebp[:].rearrange('p (r c) -> p r c', c=64),
                                ALU.bypass, 0.0, ALU.mult, vview)
                        else:
                            # ACT evicts to bf16 SBUF; the all-bf16 multiply
                            # then runs on DVE in 2x perf mode
                            eo = t * 2048 + (o % 4) * 512
                            nc.scalar.activation(A(ebS)[:, eo:eo + 512], ebp[:],
                                                 AF.Copy)
                            nc.vector.tensor_tensor(
                                out=r3(A(pB), slice(po, po + 512)),
                                in0=r3(A(ebS), slice(eo, eo + 512)),
                                in1=vview, op=ALU.mult)
                        if (k, t) in pend:
                            ppo = pend[(k, t)]
                            nc.tensor.matmul(op_ps[k][:, 512 * t:512 * t + 512],
                                             A(sw['id128']), A(pB)[:, ppo:ppo + 512],
                                             start=(o == 1), stop=False)
                        pend[(k, t)] = po
            for k in range(2):
                for t in range(2):
                    ppo = pend[(k, t)]
                    nc.tensor.matmul(op_ps[k][:, 512 * t:512 * t + 512],
                                     A(sw['id128']), A(pB)[:, ppo:ppo + 512],
                                     start=False, stop=True)
            for k in range(2):
                cs = slice(1024 * k, 1024 * k + 1024)
                stt(nc.vector, A(oS)[:, cs], op_ps[k][:], ALU.bypass, 0.0,
                    ALU.mult, A(rcpb)[:, cs])

            # out-proj + residual (block-diag + bias row)
            for k in range(2):
                ap_ps = p1k.tile([128, 1024], dt.float32, tag='t1k')
                for t in range(2):
                    nsl = slice(1024 * k + 512 * t, 1024 * k + 512 * t + 512)
                    nc.tensor.matmul(ap_ps[:, 512 * t:512 * t + 512],
                                     A(sw['outwbd']), A(oS)[:, nsl],
                                     start=True, stop=False)
                    nc.tensor.matmul(ap_ps[:, 512 * t:512 * t + 512],
                                     A(sw['outb_row']), A(sw['onesrow']),
                                     start=False, stop=True)
                rows = slice(1 + 16 * k, 1 + 16 * k + 16)
                stt(nc.vector, v3(xpad)[:, rows, 2:66],
                    ap_ps[:].rearrange('p (r c) -> p r c', c=64),
                    ALU.bypass, 0.0, ALU.add, v3(xpad)[:, rows, 2:66])

            # LN2 + ff
            layernorm_to(ynegB)
            for k in range(NT):
                nsl = slice(512 * k, 512 * k + 512)
                fp = p512.tile([128, 512], dt.float32, tag='t512')
                nc.tensor.matmul(fp[:], A(sw['ff1bd']),
                                 A(ynegB)[:, nsl], start=True, stop=True)
                nc.scalar.activation(A(gB)[:, nsl], fp[:],
                                     AF.Tanh if _GELU_SUBST_TANH else AF.Gelu)
            for k in range(2):
                f2 = p1k.tile([128, 1024], dt.float32, tag='t1k')
                for t in range(2):
                    nsl = slice(1024 * k + 512 * t, 1024 * k + 512 * t + 512)
                    nc.tensor.matmul(f2[:, 512 * t:512 * t + 512],
                                     A(sw['ff2bd']), A(gB)[:, nsl],
                                     start=True, stop=False)
                    nc.tensor.matmul(f2[:, 512 * t:512 * t + 512],
                                     A(sw['ff2b_row']), A(sw['onesrow']),
                                     start=False, stop=True)
                rows = slice(1 + 16 * k, 1 + 16 * k + 16)
                f23 = f2[:].rearrange('p (r c) -> p r c', c=64)
                stt(nc.vector, v3(xpad)[:, rows, 2:66], f23,
                    ALU.bypass, 0.0, ALU.add, v3(xpad)[:, rows, 2:66])

        nc.sync.dma_start(out=A(d_out).rearrange('p (r c) -> p r c', c=64), in_=cv3(xpad))
        stack.close()

    nc.compile()
    _BUILD_CACHE['nc'] = nc
    return nc


def kernel(**inputs):
    from concourse.bass_utils import run_bass_kernel_spmd

    x = np.asarray(inputs['x'], np.float32)
    masks = np.asarray(inputs['masks'])
    nc = _build()
    w = _pack_weights(inputs)

    in_maps = []
    for b in range(B):
        m = dict(w)
        m['x0'] = _pad_state(x[b])
        mk = masks[:, b, :, :, 0].astype(np.float32)
        mrep = np.zeros((STEPS, 128, NH), np.float32)
        for s in range(2):
            row = mk[:, 32 * s:32 * s + 32, :].reshape(STEPS, NH)
            mrep[:, 64 * s:64 * s + 64, :] = row[:, None, :]
        m['maskrep'] = mrep.astype(BF16)
        in_maps.append(m)

    import os
    trace = bool(os.environ.get('BASS_TRACE_RUN'))
    res = run_bass_kernel_spmd(nc, in_maps, core_ids=list(range(B)), trace=trace)
    if trace:
        print('exec_time_ns:', res.exec_time_ns)
    out = np.zeros((B, H, W, C), np.float32)
    for b in range(B):
        xo = np.asarray(res.results[b]['xout'], np.float32)
        for s in range(2):
            blk = xo[64 * s:64 * s + 64].reshape(64, RH, W)
            out[b, 32 * s:32 * s + 32] = np.transpose(blk, (1, 2, 0))
    return out


# revision 60
# speedup vs baseline: 1.0363x; 1.0363x over previous
"""Trainium2 Bass kernel for nn_Diffusion_ViTCA_NCA (ViT-Cellular-Automata, 6 NCA steps).

Data-parallel over batch B=8 across 8 NeuronCores (1 sample/core), fully
SBUF-resident state, feature-major half-stacked layout
[128 partitions = (2 row-halves x 64 channels), 2048 tokens-per-half].

v2 restructure vs baseline:
- conv3x3 pair + fc0 folded into one effective 3x3 conv (64 -> 128 hidden).
- block-diagonal [128]-contraction matmuls process both row-halves at once
  for qkv / out-proj / ff1 / ff2 / LN stats.
- norm0 global stats via activation accum_out (relu eviction) + DVE
  tensor_tensor_reduce (sum of squares); no separate sum matmuls.
- 1/std via ACT Ln+Exp (keeps one activation table set; no DVE reciprocal
  for LN, no Sqrt table reloads).
- attention q*k products as all-bf16 tensor_tensor (2x DVE mode); E*v
  products split DVE (half) / GPSIMD-Pool (half).
- PSUM evictions of q/k/v tiles on Pool; ch0 invariance via zeroed output
  rows in fc1/out/ff2 weights instead of restore copies.
"""

import os
import numpy as np
import ml_dtypes

# CoreSim's executor lacks Gelu; sim harnesses set this to validate the rest
# of the graph with Tanh substituted (never set in production use).
_GELU_SUBST_TANH = bool(os.environ.get('BASS_GELU_SUBST_TANH'))
# debug bisection toggles: comma list in BASS_SAFE, e.g. "pool,acc,poly,swap"
_SAFE = set(filter(None, os.environ.get('BASS_SAFE', '').split(',')))

C = 64; HID = 128; HEADS = 4; HD = 16; H = 64; W = 64; B = 8; STEPS = 6
MLP = 64; SCALE = HD ** -0.5
RH = 32            # image rows per half
PR = 34            # padded rows per half buffer
PITCH = 68         # padded col pitch
NH = RH * W        # 2048 tokens per half
NPAD = PR * PITCH  # 2312
EPS = 1e-5
BF16 = ml_dtypes.bfloat16

_BUILD_CACHE = {}


def _pad_state(xb):
    """xb [H,W,C] -> padded half-stacked state [128, PR*PITCH] fp32."""
    cf = np.transpose(np.asarray(xb, np.float32), (2, 0, 1))   # [C,H,W]
    buf = np.zeros((128, PR, PITCH), np.float32)
    for s in range(2):
        buf[64 * s:64 * s + 64, 1:33, 2:66] = cf[:, RH * s:RH * s + RH, :]
    buf[:, 1:33, 1] = buf[:, 1:33, 3]
    buf[:, 1:33, 66] = buf[:, 1:33, 64]
    buf[0:64, 0, :] = buf[0:64, 2, :]
    buf[64:128, 33, :] = buf[64:128, 31, :]
    buf[0:64, 33, :] = buf[64:128, 1, :]
    buf[64:128, 0, :] = buf[0:64, 32, :]
    return buf.reshape(128, NPAD)


def _bd(w):
    """[64out,64in] -> block-diag lhsT [128, 128] (both halves)."""
    z = np.zeros((128, 128), np.float32)
    z[0:64, 0:64] = w.T
    z[64:128, 64:128] = w.T
    return z


def _pack_weights(ip):
    f32 = np.float32
    w = {}
    assert np.allclose(np.asarray(ip['norm0_w'], f32), 1.0), "norm0_w!=1 unsupported"
    assert np.allclose(np.asarray(ip['norm0_b'], f32), 0.0), "norm0_b!=0 unsupported"
    for _bn in ('p0_b', 'p1_b', 'fc0_b', 'ff1_b', 'ln1_b', 'ln2_b'):
        assert np.allclose(np.asarray(ip[_bn], f32), 0.0), _bn + "!=0 unsupported"

    # ---- folded conv+fc0: Weff[j, i, di, dj] ----
    p0, p1 = np.asarray(ip['p0_w'], f32), np.asarray(ip['p1_w'], f32)
    fc0 = np.asarray(ip['fc0_w'], f32)      # [HID, 3C] = [x | y1 | y2]
    fc0x, fc0y1, fc0y2 = fc0[:, 0:64], fc0[:, 64:128], fc0[:, 128:192]
    cw = np.zeros((64, 9, 128), f32)        # [in-ch, offset, hid-out]
    for i in range(3):
        for j in range(3):
            o = 3 * i + j
            weff = fc0y1 @ p0[:, :, i, j] + fc0y2 @ p1[:, :, i, j]  # [HID, in]
            if i == 1 and j == 1:
                weff = weff + fc0x
            cw[:, o, :] = weff.T
    w['cweff'] = np.concatenate([cw, cw], 0).reshape(128, 9 * 128).astype(BF16)

    # ---- fc1 (ch0 of each half zeroed to keep input channel invariant) ----
    fc1 = np.asarray(ip['fc1_w'], f32).copy()
    fc1[0, :] = 0.0
    w['fc1t'] = fc1.T.copy().astype(BF16)                  # [HID, 64]
    rs = fc1.sum(1)
    w['fc1rs_row'] = np.concatenate([rs, rs]).reshape(1, 128).astype(f32)

    # ---- qkv block-diag (ln1 weight + q-scale folded) ----
    ln1w = np.asarray(ip['ln1_w'], f32)
    qkv = np.asarray(ip['qkv_w'], f32)
    qkv_eff = qkv * ln1w[None, :]
    qkv_eff[0:64] *= SCALE
    qbd = np.zeros((128, 384), f32)
    for c_ in range(3):
        qbd[:, 128 * c_:128 * c_ + 128] = _bd(qkv_eff[64 * c_:64 * c_ + 64])
    w['qkvbd'] = qbd.astype(BF16)

    # ---- out-proj block-diag, ch0 rows zeroed ----
    ow = np.asarray(ip['out_w'], f32).copy()
    ow[0, :] = 0.0
    w['outwbd'] = _bd(ow).astype(BF16)
    outb = np.asarray(ip['out_b'], f32).copy()
    outb[0] = 0.0
    w['outb_row'] = np.concatenate([outb, outb]).reshape(1, 128).astype(BF16)

    # ---- ff block-diag ----
    ln2w = np.asarray(ip['ln2_w'], f32)
    ff1 = np.asarray(ip['ff1_w'], f32) * ln2w[None, :]
    w['ff1bd'] = _bd(ff1).astype(BF16)
    ff2 = np.asarray(ip['ff2_w'], f32).copy()
    ff2[0, :] = 0.0
    w['ff2bd'] = _bd(ff2).astype(BF16)
    ff2b = np.asarray(ip['ff2_b'], f32).copy()
    ff2b[0] = 0.0
    w['ff2b_row'] = np.concatenate([ff2b, ff2b]).reshape(1, 128).astype(BF16)

    # ---- attention patterns ----
    e_of_p = (np.arange(128) // 64) * 4 + ((np.arange(128) % 64) // 16)
    dots = np.zeros((128, 9, 72), f32)
    for o in range(9):
        dots[np.arange(128), o, o * 8 + e_of_p] = 1.0
    w['dotslhs'] = dots.reshape(128, 9 * 72).astype(BF16)
    z = np.zeros((72, 8), f32)
    z[np.arange(72), np.arange(72) % 8] = 1.0
    w['zlhs'] = z.astype(BF16)
    rb = np.zeros((8, 128), f32)
    rb[e_of_p, np.arange(128)] = 1.0
    w['rcpbl'] = rb.astype(BF16)
    eb = np.zeros((72, 9, 128), f32)
    for o in range(9):
        eb[o * 8 + e_of_p, o, np.arange(128)] = 1.0
    w['eblhs'] = eb.reshape(72, 9 * 128).astype(BF16)
    w['id128'] = np.eye(128, dtype=f32).astype(BF16)
    w['onesbd'] = _bd(np.ones((64, 64), f32)).astype(BF16)
    w['ones128f'] = np.ones((128, 1), f32)
    w['ones128'] = np.ones((128, 1), f32).astype(BF16)
    w['onesrow'] = np.ones((1, 512), f32).astype(BF16)
    sw128 = np.zeros((128, 128), f32)
    sw128[np.arange(128), (np.arange(128) + 64) % 128] = 1.0
    w['swap128'] = sw128.astype(BF16)
    return w


def _wspecs(dt):
    return {
        'cweff': ([128, 9 * 128], dt.bfloat16),
        'fc1t': ([128, 64], dt.bfloat16), 'fc1rs_row': ([1, 128], dt.float32),
        'qkvbd': ([128, 384], dt.bfloat16),
        'outwbd': ([128, 128], dt.bfloat16), 'outb_row': ([1, 128], dt.bfloat16),
        'ff1bd': ([128, 128], dt.bfloat16), 'ff2bd': ([128, 128], dt.bfloat16),
        'ff2b_row': ([1, 128], dt.bfloat16),
        'dotslhs': ([128, 9 * 72], dt.bfloat16), 'zlhs': ([72, 8], dt.bfloat16),
        'rcpbl': ([8, 128], dt.bfloat16), 'eblhs': ([72, 9 * 128], dt.bfloat16),
        'id128': ([128, 128], dt.bfloat16), 'onesbd': ([128, 128], dt.bfloat16),
        'ones128f': ([128, 1], dt.float32), 'onesrow': ([1, 512], dt.bfloat16),
        'ones128': ([128, 1], dt.bfloat16), 'swap128': ([128, 128], dt.bfloat16),
    }


def _build():
    if 'nc' in _BUILD_CACHE:
        return _BUILD_CACHE['nc']
    import concourse.bass as bass
    import concourse.bacc as bacc
    import concourse.tile as tile
    from concourse import mybir
    dt = mybir.dt
    ALU = mybir.AluOpType
    AF = mybir.ActivationFunctionType
    AX = mybir.AxisListType

    nc = bacc.Bacc("TRN2", target_bir_lowering=False)
    wspecs = _wspecs(dt)

    d_x0 = nc.dram_tensor('x0', [128, NPAD], dt.float32, kind='ExternalInput')
    d_mask = nc.dram_tensor('maskrep', [STEPS, 128, NH], dt.bfloat16, kind='ExternalInput')
    d_w = {k: nc.dram_tensor(k, list(s), d, kind='ExternalInput') for k, (s, d) in wspecs.items()}
    d_out = nc.dram_tensor('xout', [128, NH], dt.float32, kind='ExternalOutput')

    A = lambda h: h.ap()

    with tile.TileContext(nc) as tc:
        xpad = nc.alloc_sbuf_tensor('xpad', [128, NPAD], dt.float32)
        xpadB = nc.alloc_sbuf_tensor('xpadB', [128, NPAD], dt.bfloat16)
        maskS = nc.alloc_sbuf_tensor('maskS', [128, STEPS * NH], dt.bfloat16)
        sw = {k: nc.alloc_sbuf_tensor('w_' + k, list(s), d) for k, (s, d) in wspecs.items()}
        hd = nc.alloc_sbuf_tensor('hd', [128, 2 * NH], dt.bfloat16)
        hdsq = nc.alloc_sbuf_tensor('hdsq', [128, 1024], dt.bfloat16)
        hd2f = (nc.alloc_sbuf_tensor('hd2f', [128, 2 * NH], dt.bfloat16)
                if 'acc' in _SAFE else None)
        tB = nc.alloc_sbuf_tensor('tB', [128, NH], dt.bfloat16)
        t2B = nc.alloc_sbuf_tensor('t2B', [128, NH], dt.bfloat16)
        uB = nc.alloc_sbuf_tensor('uB', [128, NH], dt.bfloat16)
        varB = nc.alloc_sbuf_tensor('varB', [128, NH], dt.float32)
        lnB = nc.alloc_sbuf_tensor('lnB', [128, NH], dt.float32)
        invB = nc.alloc_sbuf_tensor('invB', [128, NH], dt.float32)
        dnegB = nc.alloc_sbuf_tensor('dnegB', [128, NH], dt.float32)
        ynegB = nc.alloc_sbuf_tensor('ynegB', [128, NH], dt.bfloat16)
        qS = nc.alloc_sbuf_tensor('qS', [128, NH], dt.bfloat16)
        kpad = nc.alloc_sbuf_tensor('kpad', [128, NPAD], dt.bfloat16)
        vpadA = nc.alloc_sbuf_tensor('vpadA', [128, NPAD], dt.bfloat16)
        Ebuf = nc.alloc_sbuf_tensor('Ebuf', [72, NH], dt.bfloat16)
        e1B = nc.alloc_sbuf_tensor('e1B', [72, NH], dt.bfloat16)
        Zb = nc.alloc_sbuf_tensor('Zb', [8, NH], dt.bfloat16)
        rcpb = nc.alloc_sbuf_tensor('rcpb', [128, NH], dt.float32)
        pB = nc.alloc_sbuf_tensor('pB', [128, 4 * NH], dt.bfloat16)
        ebS = nc.alloc_sbuf_tensor('ebS', [128, 2 * NH], dt.bfloat16)
        oS = nc.alloc_sbuf_tensor('oS', [128, NH], dt.bfloat16)
        gB = nc.alloc_sbuf_tensor('gB', [128, NH], dt.bfloat16)
        tmpd = nc.alloc_sbuf_tensor('tmpd', [128, 2 * NH], dt.bfloat16)
        sc = nc.alloc_sbuf_tensor('scal', [1, 16], dt.float32)
        sc2 = nc.alloc_sbuf_tensor('scal2', [128, 8], dt.float32)
        row2 = nc.alloc_sbuf_tensor('row2', [1, 2], dt.bfloat16)
        bc3 = nc.alloc_sbuf_tensor('bc3', [128, 2], dt.float32)
        mbrow = nc.alloc_sbuf_tensor('mbrow', [1, 128], dt.bfloat16)
        epsb = nc.alloc_sbuf_tensor('epsb', [128, 1], dt.float32)
        eps2b = nc.alloc_sbuf_tensor('eps2b', [128, 1], dt.float32)

        v3 = lambda h: A(h).rearrange('p (r c) -> p r c', r=PR)
        cv3 = lambda h: v3(h)[:, 1:33, 2:66]
        r3 = lambda ap, cols: ap[:, cols].rearrange('p (r c) -> p r c', c=64)

        nc.sync.dma_start(out=A(xpad), in_=A(d_x0))
        nc.sync.dma_start(out=A(maskS).rearrange('p (s n) -> p s n', s=STEPS),
                          in_=A(d_mask).rearrange('s p n -> p s n'))
        for k in wspecs:
            nc.sync.dma_start(out=A(sw[k]), in_=A(d_w[k]))
        nc.vector.memset(A(epsb), EPS)
        nc.vector.memset(A(eps2b), 4096.0 * EPS)
        nc.vector.memset(A(kpad), 0.0)
        nc.vector.memset(A(vpadA), 0.0)

        import contextlib
        stack = contextlib.ExitStack()
        p512 = stack.enter_context(tc.tile_pool(name='p512', bufs=4, space='PSUM'))
        p1k = stack.enter_context(tc.tile_pool(name='p1k', bufs=2, space='PSUM'))

        NT = NH // 512

        def stt(eng, out, in0, op0, scalar, op1, in1):
            eng.scalar_tensor_tensor(out=out, in0=in0, scalar=scalar, in1=in1,
                                     op0=op0, op1=op1)

        def swap_halo(buf, psrc):
            """Cross-half halo rows of a padded bf16 buffer via PE half-swap
            matmul (psrc: callable row -> rhs view). Writes buf rows 0 / 33."""
            if 'swap' in _SAFE:
                nc.sync.dma_start(out=v3(buf)[0:64, 33:34, :],
                                  in_=v3(buf)[64:128, 1:2, :])
                nc.sync.dma_start(out=v3(buf)[64:128, 0:1, :],
                                  in_=v3(buf)[0:64, 32:33, :])
                return
            hp1 = p512.tile([128, 136], dt.float32, tag='t512')
            nc.tensor.matmul(hp1[:, 0:68], A(sw['swap128']), psrc(1),
                             start=True, stop=True)
            nc.tensor.matmul(hp1[:, 68:136], A(sw['swap128']), psrc(32),
                             start=True, stop=True)
            nc.scalar.activation(v3(buf)[0:64, 33:34, :], hp1[0:64, 0:68], AF.Copy)
            nc.scalar.activation(v3(buf)[64:128, 0:1, :], hp1[64:128, 68:136], AF.Copy)

        for step in range(STEPS):
            mstep = A(maskS)[:, step * NH:(step + 1) * NH]

            # bf16 shadow: interior copy, then reflect borders + cross-half
            # halos rebuilt in bf16 (fp32 xpad borders are never maintained)
            nc.scalar.activation(A(xpadB), A(xpad), AF.Copy)
            nc.scalar.activation(v3(xpadB)[:, 1:33, 1:2], v3(xpad)[:, 1:33, 3:4], AF.Copy)
            nc.scalar.activation(v3(xpadB)[:, 1:33, 66:67], v3(xpad)[:, 1:33, 64:65], AF.Copy)
            nc.scalar.activation(v3(xpadB)[0:64, 0:1, :], v3(xpad)[0:64, 2:3, :], AF.Copy)
            nc.scalar.activation(v3(xpadB)[64:128, 33:34, :], v3(xpad)[64:128, 31:32, :], AF.Copy)
            swap_halo(xpadB, lambda r: v3(xpadB)[:, r:r + 1, :])
            # warm the sqrt activation table off the critical path: reading the
            # just-written halo row anchors it after the border chain, well
            # before norm0's sqrt (scale=0 keeps sqrt in-domain)
            nc.scalar.activation(A(sc)[0:1, 15:16], v3(xpadB)[0:1, 33:34, 0:1],
                                 AF.Sqrt, scale=0.0, bias=A(epsb)[0:1, :])

            # folded conv+fc0 (9 offsets, 64ch -> 128 hid) + relu -> hd;
            # relu eviction accumulates per-partition sums for norm0.
            for k in range(2):
                for s in range(2):
                    w_idx = 2 * k + s
                    hp = p1k.tile([128, 1024], dt.float32, tag='t1k')
                    for o in range(9):
                        di, dj = o // 3 - 1, o % 3 - 1
                        for t in range(2):
                            rs = 16 * k + 8 * t
                            rhs = v3(xpadB)[64 * s:64 * s + 64,
                                            1 + di + rs:1 + di + rs + 8, 2 + dj:66 + dj]
                            nc.tensor.matmul(hp[:, 512 * t:512 * t + 512],
                                             A(sw['cweff'])[64 * s:64 * s + 64,
                                                            o * 128:(o + 1) * 128],
                                             rhs, start=(o == 0), stop=(o == 8))
                    col = s * NH + 1024 * k
                    if 'acc' in _SAFE:
                        nc.scalar.activation(A(hd)[:, col:col + 1024], hp[:], AF.Relu)
                        nc.scalar.activation(A(hd2f)[:, col:col + 1024],
                                             A(hd)[:, col:col + 1024], AF.Square)
                    else:
                        # NOTE: vector.tensor_tensor_reduce wedges the device
                        # (NRT exec-unit unrecoverable) — ACT Square+accum_out
                        # computes the sum-of-squares instead.
                        nc.scalar.activation(A(hd)[:, col:col + 1024], hp[:], AF.Relu,
                                             accum_out=A(sc2)[:, w_idx:w_idx + 1])
                        nc.scalar.activation(A(hdsq), A(hd)[:, col:col + 1024],
                                             AF.Square,
                                             accum_out=A(sc2)[:, 4 + w_idx:5 + w_idx])

            # norm0 global stats
            s1, s2 = A(sc)[:, 0:1], A(sc)[:, 1:2]
            mean, e2 = A(sc)[:, 2:3], A(sc)[:, 3:4]
            m2, var = A(sc)[:, 4:5], A(sc)[:, 5:6]
            stdv, istd, nm = A(sc)[:, 6:7], A(sc)[:, 7:8], A(sc)[:, 8:9]
            if 'acc' in _SAFE:
                sAa = p512.tile([1, 512], dt.float32, tag='t512')
                sBb = p512.tile([1, 512], dt.float32, tag='t512')
                for t in range(8):
                    nc.tensor.matmul(sAa[:], A(sw['ones128']),
                                     A(hd)[:, 512 * t:512 * t + 512],
                                     start=(t == 0), stop=(t == 7))
                for t in range(8):
                    nc.tensor.matmul(sBb[:], A(sw['ones128']),
                                     A(hd2f)[:, 512 * t:512 * t + 512],
                                     start=(t == 0), stop=(t == 7))
                nc.vector.tensor_reduce(s1, sAa[:], AX.X, ALU.add)
                nc.vector.tensor_reduce(s2, sBb[:], AX.X, ALU.add)
            else:
                sA = p512.tile([1, 8], dt.float32, tag='t512')
                nc.tensor.matmul(sA[:], A(sw['ones128f']), A(sc2)[:], start=True, stop=True)
                nc.vector.tensor_reduce(s1, sA[:, 0:4], AX.X, ALU.add)
                nc.vector.tensor_reduce(s2, sA[:, 4:8], AX.X, ALU.add)
            NTOT = 1.0 / (2 * NH * 128)
            nc.vector.tensor_scalar_mul(mean, s1, NTOT)
            nc.vector.tensor_scalar_mul(e2, s2, NTOT)
            stt(nc.vector, m2, mean, ALU.bypass, 0.0, ALU.mult, mean)
            stt(nc.vector, var, e2, ALU.bypass, 0.0, ALU.subtract, m2)
            nc.scalar.activation(stdv, var, AF.Sqrt, bias=A(epsb)[0:1, :])
            nc.vector.reciprocal_approx_fast(istd, stdv)
            nc.vector.tensor_scalar_mul(nm, mean, -1.0)
            nc.scalar.copy(A(row2)[:, 0:1], istd)
            nc.vector.tensor_scalar_mul(A(mbrow), A(sw['fc1rs_row']), nm)
            bcp = p512.tile([128, 2], dt.float32, tag='t512')
            nc.tensor.matmul(bcp[:, 0:1], A(sw['onesrow'])[:, 0:128], A(row2)[:, 0:1],
                             start=True, stop=True)
            nc.scalar.copy(A(bc3)[:, 0:1], bcp[:, 0:1])

            # fc1 (+ global-mean correction row) -> dx ; x += (dx*istd)*mask
            for k in range(2):
                dp = p1k.tile([128, 1024], dt.float32, tag='t1k')
                for t in range(2):
                    colh = 1024 * k + 512 * t
                    for half in range(2):
                        osl = (slice(64 * half, 64 * half + 64),
                               slice(512 * t, 512 * t + 512))
                        nc.tensor.matmul(dp[osl[0], osl[1]], A(sw['fc1t']),
                                         A(hd)[:, half * NH + colh:half * NH + colh + 512],
                                         start=True, stop=False)
                        nc.tensor.matmul(dp[osl[0], osl[1]],
                                         A(mbrow)[:, 64 * half:64 * half + 64],
                                         A(sw['onesrow']), start=False, stop=True)
                cs = slice(1024 * k, 1024 * k + 1024)
                tmp = A(dnegB)[:, cs]
                stt(nc.vector, tmp, dp[:], ALU.mult, A(bc3)[:, 0:1], ALU.mult, mstep[:, cs])
                rows = slice(1 + 16 * k, 1 + 16 * k + 16)
                stt(nc.vector, v3(xpad)[:, rows, 2:66], r3(A(dnegB), cs),
                    ALU.bypass, 0.0, ALU.add, v3(xpad)[:, rows, 2:66])

            def layernorm_to(dst):
                nc.scalar.activation(A(tB), cv3(xpad), AF.Copy)
                if 'pool' in _SAFE:
                    nc.scalar.activation(A(t2B), A(tB), AF.Square)
                else:
                    # x^2 on Pool straight from fp32 state (parallel with tB)
                    for kk2 in range(2):
                        c2 = slice(1024 * kk2, 1024 * kk2 + 1024)
                        rw = slice(1 + 16 * kk2, 1 + 16 * kk2 + 16)
                        nc.gpsimd.tensor_tensor(out=r3(A(t2B), c2),
                                                in0=v3(xpad)[:, rw, 2:66],
                                                in1=v3(xpad)[:, rw, 2:66], op=ALU.mult)
                for kk in range(2):
                    cs2 = slice(1024 * kk, 1024 * kk + 1024)
                    mu = p1k.tile([128, 1024], dt.float32, tag='t1k')
                    sq = p1k.tile([128, 1024], dt.float32, tag='t1k')
                    for tt in range(2):
                        nsl = slice(1024 * kk + 512 * tt, 1024 * kk + 512 * tt + 512)
                        osl = slice(512 * tt, 512 * tt + 512)
                        nc.tensor.matmul(mu[:, osl], A(sw['onesbd']),
                                         A(tB)[:, nsl], start=True, stop=True)
                        nc.tensor.matmul(sq[:, osl], A(sw['onesbd']),
                                         A(t2B)[:, nsl], start=True, stop=True)
                    nc.scalar.activation(A(uB)[:, cs2], mu[:], AF.Square)
                    stt(nc.vector, A(varB)[:, cs2], sq[:], ALU.mult, 64.0,
                        ALU.subtract, A(uB)[:, cs2])
                    nc.scalar.activation(A(lnB)[:, cs2], A(varB)[:, cs2], AF.Sqrt,
                                         bias=A(eps2b))
                    nc.vector.reciprocal_approx_fast(A(invB)[:, cs2], A(lnB)[:, cs2])
                    rows2 = slice(1 + 16 * kk, 1 + 16 * kk + 16)
                    stt(nc.vector, r3(A(dnegB), cs2), v3(xpad)[:, rows2, 2:66],
                        ALU.mult, 64.0, ALU.subtract, mu[:].rearrange('p (r c) -> p r c', c=64))
                    if 'pool' in _SAFE:
                        stt(nc.vector, A(dst)[:, cs2], A(dnegB)[:, cs2],
                            ALU.bypass, 0.0, ALU.mult, A(invB)[:, cs2])
                    else:
                        nc.gpsimd.tensor_tensor(out=A(dst)[:, cs2],
                                                in0=A(dnegB)[:, cs2],
                                                in1=A(invB)[:, cs2], op=ALU.mult)

            # LN1 + qkv (block-diagonal, both halves per matmul)
            layernorm_to(ynegB)
            for k in range(NT):
                nsl = slice(512 * k, 512 * k + 512)
                qp = p512.tile([128, 512], dt.float32, tag='t512')
                kp = p512.tile([128, 512], dt.float32, tag='t512')
                vp = p512.tile([128, 512], dt.float32, tag='t512')
                nc.tensor.matmul(qp[:], A(sw['qkvbd'])[:, 0:128],
                                 A(ynegB)[:, nsl], start=True, stop=True)
                nc.tensor.matmul(kp[:], A(sw['qkvbd'])[:, 128:256],
                                 A(ynegB)[:, nsl], start=True, stop=True)
                nc.tensor.matmul(vp[:], A(sw['qkvbd'])[:, 256:384],
                                 A(ynegB)[:, nsl], start=True, stop=True)
                rr = slice(1 + 8 * k, 1 + 8 * k + 8)
                pr3 = lambda ps: ps[:].rearrange('p (r c) -> p r c', c=64)
                nc.scalar.activation(A(qS)[:, nsl], qp[:], AF.Copy)
                nc.scalar.activation(v3(kpad)[:, rr, 2:66], pr3(kp), AF.Copy)
                nc.scalar.activation(v3(vpadA)[:, rr, 2:66], pr3(vp), AF.Copy)
            # cross-half halos for k/v via PE half-swap (no DMA latency)
            swap_halo(kpad, lambda r: v3(kpad)[:, r:r + 1, :])
            swap_halo(vpadA, lambda r: v3(vpadA)[:, r:r + 1, :])

            # dots -> pd psum; exp via 2nd-order Taylor (|logit| < 0.16):
            # e = 1 + z + z^2/2 computed as (z * (0.5 z + 1)) + 1
            for k in range(2):
                cs = slice(1024 * k, 1024 * k + 1024)
                pd = p1k.tile([72, 1024], dt.float32, tag='t1k')
                # halo-dependent offsets (di=-1 for the top band, di=+1 for
                # the bottom band) go last so dots start before the k/v halo
                # exchange completes
                oorder = [3, 4, 5, 6, 7, 8, 0, 1, 2] if k == 0 else \
                         [3, 4, 5, 0, 1, 2, 6, 7, 8]
                for oi, o in enumerate(oorder):
                    di, dj = o // 3 - 1, o % 3 - 1
                    rows = slice(1 + di + 16 * k, 1 + di + 16 * k + 16)
                    po = (oi % 2) * 1024
                    pcs = slice(po, po + 1024)
                    nc.vector.tensor_tensor(out=r3(A(tmpd), pcs), in0=r3(A(qS), cs),
                                            in1=v3(kpad)[:, rows, 2 + dj:66 + dj],
                                            op=ALU.mult)
                    for t in range(2):
                        fs = slice(512 * t, 512 * t + 512)
                        nc.tensor.matmul(pd[:, fs], A(sw['dotslhs'])[:, o * 72:(o + 1) * 72],
                                         A(tmpd)[:, po + 512 * t:po + 512 * t + 512],
                                         start=(oi == 0), stop=(oi == 8))
                if 'poly' in _SAFE:
                    nc.scalar.activation(A(Ebuf)[:, cs], pd[:], AF.Exp)
                else:
                    nc.scalar.activation(A(e1B)[:, cs], pd[:], AF.Copy,
                                         scale=0.5, bias=1.0)
                    nc.vector.tensor_tensor(out=A(Ebuf)[:, cs], in0=pd[:],
                                            in1=A(e1B)[:, cs], op=ALU.mult)
                    nc.vector.tensor_scalar(out=A(Ebuf)[:, cs], in0=A(Ebuf)[:, cs],
                                            scalar1=1.0, scalar2=None, op0=ALU.add)
                # Z and its reciprocal broadcast (needed only at the oS stage)
                zp = p1k.tile([8, 1024], dt.float32, tag='t1k')
                for t in range(2):
                    fs = slice(512 * t, 512 * t + 512)
                    nc.tensor.matmul(zp[:, fs], A(sw['zlhs']),
                                     A(Ebuf)[:, 1024 * k + 512 * t:1024 * k + 512 * t + 512],
                                     start=True, stop=True)
                nc.scalar.activation(A(Zb)[:, cs], zp[:], AF.Copy)
                zbc = p1k.tile([128, 1024], dt.float32, tag='t1k')
                for t in range(2):
                    fs = slice(512 * t, 512 * t + 512)
                    nc.tensor.matmul(zbc[:, fs], A(sw['rcpbl']),
                                     A(Zb)[:, 1024 * k + 512 * t:1024 * k + 512 * t + 512],
                                     start=True, stop=True)
                nc.vector.reciprocal_approx_fast(A(rcpb)[:, cs], zbc[:])

            # o = (sum_o Ebcast_o * v_shift_o) * rcp — k-halves interleaved:
            # E*v products stream on DVE (k=0) and Pool (k=1) in parallel
            op_ps0 = p1k.tile([128, 1024], dt.float32, tag='t1k')
            op_ps1 = p1k.tile([128, 1024], dt.float32, tag='t1k')
            op_ps = [op_ps0, op_ps1]
            pend = {}   # (k,t) -> pB offset of the previous product (PE pipelining)
            AVORD = ([3, 4, 5, 6, 7, 8, 0, 1, 2], [3, 4, 5, 0, 1, 2, 6, 7, 8])
            for oi in range(9):
                for k in range(2):
                    o = AVORD[k][oi]
                    di, dj = o // 3 - 1, o % 3 - 1
                    for t in range(2):
                        ebp = p512.tile([128, 512], dt.float32, tag='t512')
                        nc.tensor.matmul(ebp[:], A(sw['eblhs'])[:, o * 128:(o + 1) * 128],
                                         A(Ebuf)[:, 1024 * k + 512 * t:1024 * k + 512 * t + 512],
                                         start=True, stop=True)
                        po = (2 * k + t) * 2048 + (oi % 4) * 512
                        rows = slice(1 + di + 16 * k + 8 * t, 1 + di + 16 * k + 8 * t + 8)
                        vview = v3(vpadA)[:, rows, 2 + dj:66 + dj]
                        if k == 0 or 'pool' in _SAFE:
                            # DVE multiplies straight from PSUM
                            stt(nc.vector, r3(A(pB), slice(po, po + 512)),
                                ebp[:].rearrange('p (r c) -> p r c', c=64),
                                ALU.bypass, 0.0, ALU.mult, vview)
                        else:
                            # ACT evicts to bf16 SBUF; the all-bf16 multiply
                            # then runs on DVE in 2x perf mode
                            eo = t * 2048 + (oi % 4) * 512
                            nc.scalar.activation(A(ebS)[:, eo:eo + 512], ebp[:],
                                                 AF.Copy)
                            nc.vector.tensor_tensor(
                                out=r3(A(pB), slice(po, po + 512)),
                                in0=r3(A(ebS), slice(eo, eo + 512)),
                                in1=vview, op=ALU.mult)
                        if (k, t) in pend:
                            ppo = pend[(k, t)]
                            nc.tensor.matmul(op_ps[k][:, 512 * t:512 * t + 512],
                                             A(sw['id128']), A(pB)[:, ppo:ppo + 512],
                                             start=(oi == 1), stop=False)
                        pend[(k, t)] = po
            for k in range(2):
                for t in range(2):
                    ppo = pend[(k, t)]
                    nc.tensor.matmul(op_ps[k][:, 512 * t:512 * t + 512],
                                     A(sw['id128']), A(pB)[:, ppo:ppo + 512],
                                     start=False, stop=True)
            for k in range(2):
                cs = slice(1024 * k, 1024 * k + 1024)
                stt(nc.vector, A(oS)[:, cs], op_ps[k][:], ALU.bypass, 0.0,
                    ALU.mult, A(rcpb)[:, cs])

            # out-proj + residual (block-diag + bias row)
            for k in range(2):
                ap_ps = p1k.tile([128, 1024], dt.float32, tag='t1k')
                for t in range(2):
                    nsl = slice(1024 * k + 512 * t, 1024 * k + 512 * t + 512)
                    nc.tensor.matmul(ap_ps[:, 512 * t:512 * t + 512],
                                     A(sw['outwbd']), A(oS)[:, nsl],
                                     start=True, stop=False)
                    nc.tensor.matmul(ap_ps[:, 512 * t:512 * t + 512],
                                     A(sw['outb_row']), A(sw['onesrow']),
                                     start=False, stop=True)
                rows = slice(1 + 16 * k, 1 + 16 * k + 16)
                stt(nc.vector, v3(xpad)[:, rows, 2:66],
                    ap_ps[:].rearrange('p (r c) -> p r c', c=64),
                    ALU.bypass, 0.0, ALU.add, v3(xpad)[:, rows, 2:66])

            # LN2 + ff
            layernorm_to(ynegB)
            for k in range(NT):
                nsl = slice(512 * k, 512 * k + 512)
                fp = p512.tile([128, 512], dt.float32, tag='t512')
                nc.tensor.matmul(fp[:], A(sw['ff1bd']),
                                 A(ynegB)[:, nsl], start=True, stop=True)
                nc.scalar.activation(A(gB)[:, nsl], fp[:],
                                     AF.Tanh if _GELU_SUBST_TANH else AF.Gelu)
            for k in range(2):
                f2 = p1k.tile([128, 1024], dt.float32, tag='t1k')
                for t in range(2):
                    nsl = slice(1024 * k + 512 * t, 1024 * k + 512 * t + 512)
                    nc.tensor.matmul(f2[:, 512 * t:512 * t + 512],
                                     A(sw['ff2bd']), A(gB)[:, nsl],
                                     start=True, stop=False)
                    nc.tensor.matmul(f2[:, 512 * t:512 * t + 512],
                                     A(sw['ff2b_row']), A(sw['onesrow']),
                                     start=False, stop=True)
                rows = slice(1 + 16 * k, 1 + 16 * k + 16)
                f23 = f2[:].rearrange('p (r c) -> p r c', c=64)
                stt(nc.vector, v3(xpad)[:, rows, 2:66], f23,
                    ALU.bypass, 0.0, ALU.add, v3(xpad)[:, rows, 2:66])

        nc.sync.dma_start(out=A(d_out).rearrange('p (r c) -> p r c', c=64), in_=cv3(xpad))
        stack.close()

    nc.compile()
    _BUILD_CACHE['nc'] = nc
    return nc


def kernel(**inputs):
    from concourse.bass_utils import run_bass_kernel_spmd

    x = np.asarray(inputs['x'], np.float32)
    masks = np.asarray(inputs['masks'])
    nc = _build()
    w = _pack_weights(inputs)

    in_maps = []
    for b in range(B):
        m = dict(w)
        m['x0'] = _pad_state(x[b])
        mk = masks[:, b, :, :, 0].astype(np.float32)
        mrep = np.zeros((STEPS, 128, NH), np.float32)
        for s in range(2):
            row = mk[:, 32 * s:32 * s + 32, :].reshape(STEPS, NH)
            mrep[:, 64 * s:64 * s + 64, :] = row[:, None, :]
        m['maskrep'] = mrep.astype(BF16)
        in_maps.append(m)

    import os
    trace = bool(os.environ.get('BASS_TRACE_RUN'))
    res = run_bass_kernel_spmd(nc, in_maps, core_ids=list(range(B)), trace=trace)
    if trace:
        print('exec_time_ns:', res.exec_time_ns)
    out = np.zeros((B, H, W, C), np.float32)
    for b in range(B):
        xo = np.asarray(res.results[b]['xout'], np.float32)
        for s in range(2):
            blk = xo[64 * s:64 * s + 64].reshape(64, RH, W)
            out[b, 32 * s:32 * s + 32] = np.transpose(blk, (1, 2, 0))
    return out
